# revision 10
# baseline (speedup 1.0000x reference)
"""Multi-head attention (B=2, S=2048, D=1024, H=16, hd=64) with RoPE on 8 TRN2
NeuronCores.

Sharding: 2 batches x 4 head-groups. Core c handles batch c//4, heads
[4*(c%4), 4*(c%4)+4). Each core computes Q/K/V projections for its heads from
the full sequence, RoPE, unnormalized attention (exp(q.k/8) streamed through
PSUM with an exp(mask) column appended to V to collect the softmax row sums),
then normalizes. Loop order is q-tile outer / head-pair inner so that the
partial output projection for each 512-row q-tile completes mid-kernel and its
ReduceScatter (over the batch's 4-core group) overlaps the remaining
attention. The host reassembles the 4x4 (qtile, rank) x 128-row slices and
adds the (wo + wv@wo) bias.

Layout notes:
- x is uploaded pre-transposed (xT [D, S]) so it serves both as matmul rhs for
  Q^T/K^T production and as lhsT for V production.
- Q^T/K^T rows within each head are permuted to (d0,d32,d1,d33,...) so the
  RoPE partner lives in the adjacent partition; a stream_shuffle with the
  pair-swap mask plus two multiplies by host-precomputed cos/sin tables
  implements the rotation with all operands partition-aligned. The score
  matmul contracts over the permuted axis, which is permutation-invariant as
  long as Q and K share the ordering.
- The attention mask enters as exp(mask[k]) multiplied into V's rows (and
  the appended row-sum column), which is exact and free.
- Softmax row-sum reciprocals are folded via DMA into a [128, n] layout so
  the DVE divide runs on all lanes, then broadcast back via a DRAM round
  trip on the gpsimd queue.
"""

import numpy as np
import ml_dtypes

import concourse.bass as bass
import concourse.mybir as mybir
from concourse import bacc, bass_utils
import concourse.tile as tile

B, S, DIM, HEADS, HD = 2, 2048, 1024, 16, 64
HPC = HEADS // 4          # heads per core = 4
P = 128
NKC = DIM // P            # 8 contraction chunks for projections
NSC = S // P              # 16 sequence chunks of 128
NQT = S // 512            # 4 q tiles of 512
SQ = S // 4               # 512-row output slice per core
VW = HPC * (HD + 1)       # 260: V with a row-sum column per head
fp32 = mybir.dt.float32
bf16 = mybir.dt.bfloat16

_CACHE = {}


def _build(dbg=False):
    nc = bacc.Bacc("TRN2", target_bir_lowering=False, debug=False, num_devices=8)

    xT = nc.dram_tensor("xT", [DIM, S], bf16, kind="ExternalInput")
    wq = nc.dram_tensor("wq", [DIM, HPC * HD], bf16, kind="ExternalInput")
    wk = nc.dram_tensor("wk", [DIM, HPC * HD], bf16, kind="ExternalInput")
    wv = nc.dram_tensor("wv", [DIM, HPC * HD], bf16, kind="ExternalInput")
    wo = nc.dram_tensor("wo", [HPC * HD, DIM], bf16, kind="ExternalInput")
    trigA = nc.dram_tensor("trigA", [P, S], bf16, kind="ExternalInput")
    trigB = nc.dram_tensor("trigB", [P, S], bf16, kind="ExternalInput")
    qbias = nc.dram_tensor("qbias", [P, 2], fp32, kind="ExternalInput")
    kbias = nc.dram_tensor("kbias", [P, 2], fp32, kind="ExternalInput")
    em = nc.dram_tensor("em", [P, NSC], fp32, kind="ExternalInput")
    out = nc.dram_tensor("out", [NQT * P, DIM], bf16, kind="ExternalOutput")

    SWAP_MASK = [i ^ 1 for i in range(32)]

    with tile.TileContext(nc) as tc:
        with (
            tc.tile_pool(name="const", bufs=1) as const,
            tc.tile_pool(name="work", bufs=3) as work,
            tc.tile_pool(name="attp", bufs=6) as attp,
            tc.tile_pool(name="stun", bufs=10) as stun,
            tc.tile_pool(name="ps_proj", bufs=2, space="PSUM") as ps_proj,
            tc.tile_pool(name="ps_o", bufs=2, space="PSUM") as ps_o,
            tc.tile_pool(name="ps_sT", bufs=2, space="PSUM") as ps_sT,
            tc.tile_pool(name="dram", bufs=1, space="DRAM") as dram,
            tc.tile_pool(name="dram_rc", bufs=3, space="DRAM") as dram_rc,
        ):
            # ---- load constants / inputs into SBUF ----
            # wq first (small, needed for the first projection matmul), then
            # xT chunks alternating between the sync and scalar HWDGE queues
            # so two uploads stream in parallel.
            wq_sb = const.tile([P, NKC, HPC * HD], bf16)
            nc.sync.dma_start(wq_sb[:], wq.rearrange("(c p) m -> p c m", p=P))
            wk_sb = const.tile([P, NKC, HPC * HD], bf16)
            nc.scalar.dma_start(wk_sb[:], wk.rearrange("(c p) m -> p c m", p=P))
            xT_sb = const.tile([P, NKC, S], bf16)
            xT_r = xT.rearrange("(c p) s -> p c s", p=P)
            for kc in range(NKC):
                eng = nc.sync if kc % 2 == 0 else nc.scalar
                eng.dma_start(xT_sb[:, kc, :], xT_r[:, kc, :])
            qb_sb = const.tile([P, 2], fp32)
            nc.sync.dma_start(qb_sb[:], qbias[:])
            kb_sb = const.tile([P, 2], fp32)
            nc.sync.dma_start(kb_sb[:], kbias[:])
            em_sb = const.tile([P, NSC], fp32)
            nc.sync.dma_start(em_sb[:], em[:])
            wv_sb = const.tile([P, NKC, HPC * HD], bf16)
            nc.scalar.dma_start(wv_sb[:], wv.rearrange("(c p) m -> p c m", p=P))
            trigA_sb = const.tile([P, S], bf16)
            nc.sync.dma_start(trigA_sb[:], trigA[:])
            trigB_sb = const.tile([P, S], bf16)
            nc.scalar.dma_start(trigB_sb[:], trigB[:])
            wo_sb = const.tile([P, 2, DIM], bf16)
            nc.sync.dma_start(wo_sb[:], wo.rearrange("(c p) m -> p c m", p=P))

            warm_in = dram.tile([P, 4], fp32, name="warm_in")
            warm_out = dram.tile([P, 4], fp32, name="warm_out")
            wz = work.tile([P, 4], fp32, tag="wz", name="wz")
            nc.vector.memset(wz[:], 0.0)
            nc.gpsimd.dma_start(warm_in[:], wz[:])
            nc.gpsimd.collective_compute(
                "AllReduce", mybir.AluOpType.add,
                replica_groups=[[0, 1, 2, 3], [4, 5, 6, 7]],
                ins=[warm_in.opt()], outs=[warm_out.opt()],
            )
            wrs_in = dram.tile([512, DIM], bf16, name="wrs_in")
            wrs_out = dram.tile([P, DIM], bf16, name="wrs_out")
            nc.gpsimd.collective_compute(
                "ReduceScatter", mybir.AluOpType.add,
                replica_groups=[[0, 1, 2, 3], [4, 5, 6, 7]],
                ins=[wrs_in.opt()], outs=[wrs_out.opt()],
            )

            QT_rot = const.tile([P, 2, S], bf16)   # heads 0,1 | 2,3 stacked
            KT_rot = const.tile([P, 2, S], bf16)
            V_aug = const.tile([P, NSC, VW], bf16)  # [s-chunk, 4*(64+1)]

            # RoPE: bias add on ACT (idle in prelude) or DVE, then shuffle,
            # two trig muls, add on DVE.
            def rope_chain(pss_sc, b_sb, dst, cq, sc, on_scalar=True):
                q_sb = work.tile([P, 512], bf16, tag="q_sb",
                                 name=f"q_sb_{cq}_{sc}")
                if on_scalar:
                    nc.scalar.add(q_sb[:], pss_sc[:], b_sb[:, cq:cq + 1])
                else:
                    nc.vector.tensor_scalar_add(
                        q_sb[:], pss_sc[:], b_sb[:, cq:cq + 1])
                q_sw = work.tile([P, 512], bf16, tag="q_sw",
                                 name=f"q_sw_{cq}_{sc}")
                nc.vector.stream_shuffle(q_sw[:], q_sb[:], SWAP_MASK)
                p1 = work.tile([P, 512], bf16, tag="p1", name=f"p1_{cq}_{sc}")
                nc.vector.tensor_mul(
                    p1[:], q_sb[:], trigA_sb[:, sc * 512:(sc + 1) * 512])
                p2 = work.tile([P, 512], bf16, tag="p2", name=f"p2_{cq}_{sc}")
                nc.vector.tensor_mul(
                    p2[:], q_sw[:], trigB_sb[:, sc * 512:(sc + 1) * 512])
                nc.vector.tensor_add(
                    dst[:, cq, sc * 512:(sc + 1) * 512], p1[:], p2[:])

            # ---- Q^T cq0 / K^T cq0+cq1 projections + RoPE (prelude) ----
            # kc-outer so each xT chunk is consumed as soon as its DMA lands.
            # 4 sequence tiles accumulate at once: 2 from ps_proj, 2 from
            # ps_sT (borrowed; attention hasn't started).
            for w_sb, b_sb, dst, cq in (
                (wq_sb, qb_sb, QT_rot, 0),
                (wk_sb, kb_sb, KT_rot, 0),
                (wk_sb, kb_sb, KT_rot, 1),
            ):
                pss = []
                for sc in range(4):
                    pool = ps_proj if sc < 2 else ps_sT
                    tag = "proj" if sc < 2 else "sT"
                    pss.append(pool.tile([P, 512], fp32, tag=tag,
                                         name=f"pss_{cq}_{sc}_{dst is KT_rot}"))
                for kc in range(NKC):
                    for sc in range(4):
                        nc.tensor.matmul(
                            pss[sc][:],
                            w_sb[:, kc, cq * P:(cq + 1) * P],
                            xT_sb[:, kc, sc * 512:(sc + 1) * 512],
                            start=(kc == 0), stop=(kc == NKC - 1),
                        )
                for sc in range(4):
                    rope_chain(pss[sc], b_sb, dst, cq, sc)

            # ---- V projection chunk (natural layout, scaled by exp(mask)) ----
            def v_chunk(sc):
                ps = ps_proj.tile([P, HPC * HD], fp32, tag="proj",
                                  name=f"vp_{sc}")
                for kc in range(NKC):
                    nc.tensor.matmul(
                        ps[:],
                        xT_sb[:, kc, sc * P:(sc + 1) * P],
                        wv_sb[:, kc, :],
                        start=(kc == 0), stop=(kc == NKC - 1),
                    )
                # per head: columns 0..63 = V * exp(mask), column 64 = exp(mask)
                vdst = V_aug[:, sc, :].rearrange("p (h x) -> p h x", h=HPC)
                nc.scalar.mul(
                    vdst[:, :, 0:HD],
                    ps[:].rearrange("p (h x) -> p h x", h=HPC),
                    em_sb[:, sc:sc + 1],
                )
                nc.vector.tensor_copy(
                    vdst[:, :, HD:HD + 1],
                    em_sb[:, sc:sc + 1, None].to_broadcast([P, HPC, 1]),
                )

            for sc in range(10):
                v_chunk(sc)

            # deferred work executed inside attention iterations:
            #  - remaining V chunks (10..15) early in (qt=0, hp=0); each is
            #    produced >=8 iterations before its attn_v consumer
            #  - Q cq=1 projection for tile qt during (qt, hp=0)
            defer_pss = {}

            def defer_v(it):
                if it < 6:
                    v_chunk(10 + it)

            def defer_q1(qt, m):
                # 8 matmuls (kc 0..7) over the first 8 iterations + rope
                if m < NKC:
                    kc = m
                    if kc == 0:
                        defer_pss[qt] = ps_proj.tile(
                            [P, 512], fp32, tag="proj", name=f"ip_{qt}")
                    nc.tensor.matmul(
                        defer_pss[qt][:],
                        wq_sb[:, kc, P:2 * P],
                        xT_sb[:, kc, qt * 512:(qt + 1) * 512],
                        start=(kc == 0), stop=(kc == NKC - 1),
                    )
                if m == NKC:
                    rope_chain(defer_pss[qt], qb_sb, QT_rot, 1, qt,
                               on_scalar=False)

            # ---- attention: q-tile outer, head pair inner ----
            oT_norm = const.tile([P, 2, S], bf16)   # normalized o^T, heads packed
            cc_in = [dram.tile([512, DIM], bf16, name=f"cc_in_{qt}")
                     for qt in range(NQT)]
            cc_out = [dram.tile([P, DIM], bf16, name=f"cc_out_{qt}")
                      for qt in range(NQT)]
            rs_fold = const.tile([P, NQT, 2, 2, 4], bf16)  # row sums [qt, hp, j]

            def oproj(qt):
                # partial output projection for rows [512qt, 512qt+512)
                for qs in range(4):
                    r0 = qt * 512 + qs * P
                    o_sb = work.tile([P, DIM], bf16, tag="o_sb",
                                     name=f"osb_{qt}_{qs}")
                    for dc in range(2):
                        ps = ps_proj.tile([P, 512], fp32, tag="proj",
                                          name=f"op_{qt}_{qs}_{dc}")
                        for c in range(2):
                            nc.tensor.matmul(
                                ps[:],
                                oT_norm[:, c, r0:r0 + P],
                                wo_sb[:, c, dc * 512:(dc + 1) * 512],
                                start=(c == 0), stop=(c == 1),
                            )
                        nc.vector.tensor_copy(
                            o_sb[:, dc * 512:(dc + 1) * 512], ps[:])
                    nc.scalar.dma_start(
                        cc_in[qt][qs * P:(qs + 1) * P, :], o_sb[:])
                nc.gpsimd.collective_compute(
                    "ReduceScatter", mybir.AluOpType.add,
                    replica_groups=[[0, 1, 2, 3], [4, 5, 6, 7]],
                    ins=[cc_in[qt].opt()], outs=[cc_out[qt].opt()],
                )
                nc.gpsimd.dma_start(out[qt * P:(qt + 1) * P, :], cc_out[qt][:])

            for qt in range(NQT):
                for hp in range(2):
                    oTs = [ps_o.tile([HD + 1, 512], fp32, tag="oT",
                                     name=f"oT_{qt}_{hp}_{j}")
                           for j in range(2)]

                    def attn_v(kb, at_kb):
                        for j in range(2):
                            h = 2 * hp + j
                            nc.tensor.matmul(
                                oTs[j][:],
                                V_aug[:, kb, h * (HD + 1):(h + 1) * (HD + 1)],
                                at_kb[:, j, :],
                                start=(kb == 0), stop=(kb == NSC - 1),
                            )

                    # attnV software-pipelined two steps behind the exp
                    # stream so the PE never waits on the current tile's exp
                    pend = []
                    for kb in range(NSC):
                        sT = ps_sT.tile([P, 2, 512], fp32, tag="sT")
                        # the two heads occupy partitions 0-63 / 64-127, so
                        # the two K=64 score matmuls run concurrently
                        for j in range(2):
                            nc.tensor.matmul(
                                sT[:, j, :],
                                KT_rot[64 * j:64 * j + 64, hp, kb * P:(kb + 1) * P],
                                QT_rot[64 * j:64 * j + 64, hp,
                                       qt * 512:(qt + 1) * 512],
                                start=True, stop=True,
                            )
                        at = attp.tile([P, 2, 512], bf16, tag="attnT")
                        nc.scalar.activation(
                            at[:], sT[:], mybir.ActivationFunctionType.Exp,
                            scale=0.125)
                        if qt == 0:
                            defer_v(16 * hp + kb)
                        if hp == 0:
                            defer_q1(qt, kb)
                        pend.append((kb, at))
                        if len(pend) > 2:
                            attn_v(*pend.pop(0))
                    for p_ in pend:
                        attn_v(*p_)

                    # stage the head values off PSUM (frees the banks) and
                    # fold the row sums into [128, 4] lanes via DMA
                    st_un = {}
                    for j in range(2):
                        st = stun.tile([HD + 1, 512], bf16, tag="st_un",
                                       name=f"st_un_{qt}_{hp}_{j}")
                        nc.vector.tensor_copy(st[:], oTs[j][:])
                        nc.sync.dma_start(
                            rs_fold[:, qt, hp, j, :], st[HD:HD + 1, :])
                        st_un[j] = st
                    # reciprocal on all 128 lanes at once, then broadcast
                    # back via DRAM on the gpsimd queue
                    rc = work.tile([P, 2, 4], bf16, tag="rc2",
                                   name=f"rc_{qt}_{hp}")
                    with nc.allow_low_precision(
                            reason="softmax scale in bf16 is within budget"):
                        nc.vector.reciprocal(rc[:], rs_fold[:, qt, hp, :, :])
                    rcd = dram_rc.tile([2, 512], bf16, tag="rcd",
                                       name=f"rcd_{qt}_{hp}")
                    for j in range(2):
                        nc.sync.dma_start(rcd[j:j + 1, :], rc[:, j, :])
                    for j in range(2):
                        pbase = 64 * j
                        rb = work.tile([HD, 512], bf16, tag="rbcast")
                        rsrc = rcd[j:j + 1, :]
                        nc.sync.dma_start(
                            rb[:],
                            bass.AP(rsrc.tensor, rsrc.offset, [[0, HD], [1, 512]]))
                        stage = work.tile([HD, 512], bf16, tag="stage")
                        nc.vector.tensor_mul(stage[:], st_un[j][0:HD, :], rb[:])
                        nc.sync.dma_start(
                            oT_norm[pbase:pbase + 64, hp,
                                    qt * 512:(qt + 1) * 512],
                            stage[:])
                oproj(qt)

    nc.compile()
    return nc


def _host_prep(x, pos, mask, wq_kernel, wq_bias, wk_kernel, wk_bias,
               wv_kernel, wv_bias, wo_kernel, wo_bias):
    """Build per-core in_maps for the 8 cores."""
    perm = np.array([(j // 2) if j % 2 == 0 else (j // 2 + 32)
                     for j in range(HD)])
    half = HD // 2
    freqs = (10000.0 ** (-np.linspace(0.0, 1.0, half, endpoint=False))).astype(np.float64)

    bf = ml_dtypes.bfloat16
    in_maps = []
    for c in range(8):
        b, g = c // 4, c % 4
        H = list(range(HPC * g, HPC * g + HPC))

        theta = pos[b].astype(np.float64)[:, None] * freqs[None, :]  # [S, 32]
        cos = np.cos(theta).astype(np.float32)
        sin = np.sin(theta).astype(np.float32)
        trigA = np.empty((P, S), np.float32)
        trigB = np.empty((P, S), np.float32)
        for r in range(P):
            j = r % HD
            i = j // 2
            trigA[r] = cos[:, i]
            trigB[r] = (-sin[:, i]) if j % 2 == 0 else sin[:, i]

        def permute_w(wk_):  # [D, H, hd] -> [D, 4*64] with rope-pair row order
            wsel = wk_[:, H, :][:, :, perm]          # [D, 4, 64]
            return np.ascontiguousarray(wsel.reshape(DIM, HPC * HD))

        def permute_b(bias):  # [H, hd] -> [128, 2]
            bsel = bias[H][:, perm]                  # [4, 64]
            return np.ascontiguousarray(bsel.reshape(2, P).T)

        emv = np.exp(mask[b, 0, 0].astype(np.float32))  # [S]

        in_maps.append({
            "xT": np.ascontiguousarray(x[b].T).astype(bf),
            "wq": permute_w(wq_kernel).astype(bf),
            "wk": permute_w(wk_kernel).astype(bf),
            "wv": np.ascontiguousarray(
                wv_kernel[:, H, :].reshape(DIM, HPC * HD)).astype(bf),
            "wo": np.ascontiguousarray(
                wo_kernel[H].reshape(HPC * HD, DIM)).astype(bf),
            "trigA": trigA.astype(bf),
            "trigB": trigB.astype(bf),
            "qbias": permute_b(wq_bias),
            "kbias": permute_b(wk_bias),
            "em": np.ascontiguousarray(emv.reshape(NSC, P).T),
        })
    return in_maps


def kernel(x, pos, mask, wq_kernel, wq_bias, wk_kernel, wk_bias,
           wv_kernel, wv_bias, wo_kernel, wo_bias):
    x, pos, mask = np.asarray(x), np.asarray(pos), np.asarray(mask)
    wq_kernel, wq_bias = np.asarray(wq_kernel), np.asarray(wq_bias)
    wk_kernel, wk_bias = np.asarray(wk_kernel), np.asarray(wk_bias)
    wv_kernel, wv_bias = np.asarray(wv_kernel), np.asarray(wv_bias)
    wo_kernel, wo_bias = np.asarray(wo_kernel), np.asarray(wo_bias)
    if "nc" not in _CACHE:
        _CACHE["nc"] = _build()
    nc = _CACHE["nc"]

    in_maps = _host_prep(x, pos, mask, wq_kernel, wq_bias, wk_kernel, wk_bias,
                         wv_kernel, wv_bias, wo_kernel, wo_bias)
    res = bass_utils.run_bass_kernel_spmd(
        nc, in_maps, core_ids=list(range(8)))

    final_bias = (wo_bias.astype(np.float64)
                  + np.einsum("hd,hdo->o", wv_bias.astype(np.float64),
                              wo_kernel.astype(np.float64))).astype(np.float32)

    # out rows on core (b, g), chunk qt are batch-b rows [512qt+128g, +128)
    outs = []
    for b in range(B):
        rows = np.empty((S, DIM), np.float32)
        for g in range(4):
            o = np.asarray(res.results[4 * b + g]["out"]).astype(np.float32)
            for qt in range(NQT):
                r0 = 512 * qt + P * g
                rows[r0:r0 + P] = o[qt * P:(qt + 1) * P]
        outs.append(rows + final_bias[None, :])
    return np.stack(outs, axis=0)


# revision 13
# speedup vs baseline: 1.1607x; 1.1607x over previous
"""Multi-head attention (B=2, S=2048, D=1024, H=16, hd=64) with RoPE on 8 TRN2
NeuronCores.

Sharding: 2 batches x 4 head-groups. Core c handles batch c//4, heads
[4*(c%4), 4*(c%4)+4). Each core computes Q/K/V projections for its heads from
the full sequence, RoPE, unnormalized attention (exp(q.k/8) streamed through
PSUM with an exp(mask) column appended to V to collect the softmax row sums),
then normalizes. Loop order is q-tile outer / head-pair inner so that the
partial output projection for each 512-row q-tile completes mid-kernel and its
ReduceScatter (over the batch's 4-core group) overlaps the remaining
attention. The host reassembles the 4x4 (qtile, rank) x 128-row slices and
adds the (wo + wv@wo) bias.

Layout notes:
- x is uploaded pre-transposed (xT [D, S]) so it serves both as matmul rhs for
  Q^T/K^T production and as lhsT for V production.
- Q^T/K^T rows within each head are permuted to (d0,d32,d1,d33,...) so the
  RoPE partner lives in the adjacent partition; a stream_shuffle with the
  pair-swap mask plus two multiplies by host-precomputed cos/sin tables
  implements the rotation with all operands partition-aligned. The score
  matmul contracts over the permuted axis, which is permutation-invariant as
  long as Q and K share the ordering.
- The attention mask enters as exp(mask[k]) multiplied into V's rows (and
  the appended row-sum column), which is exact and free.
- Softmax row-sum reciprocals are folded via DMA into a [128, n] layout so
  the DVE divide runs on all lanes, then broadcast back via a DRAM round
  trip on the gpsimd queue.
"""

import numpy as np
import ml_dtypes

import concourse.bass as bass
import concourse.mybir as mybir
from concourse import bacc, bass_utils
import concourse.tile as tile

B, S, DIM, HEADS, HD = 2, 2048, 1024, 16, 64
HPC = HEADS // 4          # heads per core = 4
P = 128
NKC = DIM // P            # 8 contraction chunks for projections
NSC = S // P              # 16 sequence chunks of 128
NQT = S // 512            # 4 q tiles of 512
SQ = S // 4               # 512-row output slice per core
VW = HPC * (HD + 1)       # 260: V with a row-sum column per head
fp32 = mybir.dt.float32
bf16 = mybir.dt.bfloat16

_CACHE = {}


def _build(dbg=False):
    nc = bacc.Bacc("TRN2", target_bir_lowering=False, debug=False, num_devices=8)

    xT = nc.dram_tensor("xT", [DIM, S], bf16, kind="ExternalInput")
    wq = nc.dram_tensor("wq", [DIM, HPC * HD], bf16, kind="ExternalInput")
    wk = nc.dram_tensor("wk", [DIM, HPC * HD], bf16, kind="ExternalInput")
    wv = nc.dram_tensor("wv", [DIM, HPC * HD], bf16, kind="ExternalInput")
    wo = nc.dram_tensor("wo", [P, 8 * DIM], bf16, kind="ExternalInput")
    trigA = nc.dram_tensor("trigA", [P, S], bf16, kind="ExternalInput")
    trigB = nc.dram_tensor("trigB", [P, S], bf16, kind="ExternalInput")
    qbias = nc.dram_tensor("qbias", [P, 2], fp32, kind="ExternalInput")
    kbias = nc.dram_tensor("kbias", [P, 2], fp32, kind="ExternalInput")
    em = nc.dram_tensor("em", [P, NSC], fp32, kind="ExternalInput")
    out = nc.dram_tensor("out", [NQT * P, DIM], bf16, kind="ExternalOutput")

    SWAP_MASK = [i ^ 1 for i in range(32)]

    with tile.TileContext(nc) as tc:
        with (
            tc.tile_pool(name="const", bufs=1) as const,
            tc.tile_pool(name="work", bufs=3) as work,
            tc.tile_pool(name="attp", bufs=6) as attp,
            tc.tile_pool(name="stun", bufs=10) as stun,
            tc.tile_pool(name="ps_proj", bufs=2, space="PSUM") as ps_proj,
            tc.tile_pool(name="ps_o", bufs=2, space="PSUM") as ps_o,
            tc.tile_pool(name="ps_sT", bufs=2, space="PSUM") as ps_sT,
            tc.tile_pool(name="dram", bufs=1, space="DRAM") as dram,
            tc.tile_pool(name="dram_rc", bufs=3, space="DRAM") as dram_rc,
        ):
            # ---- load constants / inputs into SBUF ----
            # wq first (small, needed for the first projection matmul), then
            # xT chunks alternating between the sync and scalar HWDGE queues
            # so two uploads stream in parallel.
            wq_sb = const.tile([P, NKC, HPC * HD], bf16)
            nc.sync.dma_start(wq_sb[:], wq.rearrange("(c p) m -> p c m", p=P))
            wk_sb = const.tile([P, NKC, HPC * HD], bf16)
            nc.scalar.dma_start(wk_sb[:], wk.rearrange("(c p) m -> p c m", p=P))
            xT_sb = const.tile([P, NKC, S], bf16)
            xT_r = xT.rearrange("(c p) s -> p c s", p=P)
            for kc in range(NKC):
                eng = nc.sync if kc % 2 == 0 else nc.scalar
                eng.dma_start(xT_sb[:, kc, :], xT_r[:, kc, :])
            qb_sb = const.tile([P, 2], fp32)
            nc.sync.dma_start(qb_sb[:], qbias[:])
            kb_sb = const.tile([P, 2], fp32)
            nc.sync.dma_start(kb_sb[:], kbias[:])
            em_sb = const.tile([P, NSC], fp32)
            nc.sync.dma_start(em_sb[:], em[:])
            wv_sb = const.tile([P, NKC, HPC * HD], bf16)
            nc.scalar.dma_start(wv_sb[:], wv.rearrange("(c p) m -> p c m", p=P))
            trigA_sb = const.tile([P, S], bf16)
            nc.sync.dma_start(trigA_sb[:], trigA[:])
            trigB_sb = const.tile([P, S], bf16)
            nc.scalar.dma_start(trigB_sb[:], trigB[:])
            wo_sb = const.tile([P, 8, DIM], bf16)
            nc.sync.dma_start(wo_sb[:], wo[:].rearrange("p (g m) -> p g m", g=8))

            warm_in = dram.tile([P, 4], fp32, name="warm_in")
            warm_out = dram.tile([P, 4], fp32, name="warm_out")
            wz = work.tile([P, 4], fp32, tag="wz", name="wz")
            nc.vector.memset(wz[:], 0.0)
            nc.gpsimd.dma_start(warm_in[:], wz[:])
            nc.gpsimd.collective_compute(
                "AllReduce", mybir.AluOpType.add,
                replica_groups=[[0, 1, 2, 3], [4, 5, 6, 7]],
                ins=[warm_in.opt()], outs=[warm_out.opt()],
            )
            wa_in = dram.tile([P, 4], fp32, name="wa_in")
            wa_out = dram.tile([P, 4], fp32, name="wa_out")
            nc.gpsimd.dma_start(wa_in[:], wz[:])
            nc.gpsimd.collective_compute(
                "AllToAll", mybir.AluOpType.bypass,
                replica_groups=[[0, 1, 2, 3, 4, 5, 6, 7]],
                ins=[wa_in.opt()], outs=[wa_out.opt()],
            )

            QT_rot = const.tile([P, 2, S], bf16)   # heads 0,1 | 2,3 stacked
            KT_rot = const.tile([P, 2, S], bf16)
            V_aug = const.tile([P, NSC, VW], bf16)  # [s-chunk, 4*(64+1)]

            # RoPE: bias add on ACT (idle in prelude) or DVE, then shuffle,
            # two trig muls, add on DVE.
            def rope_chain(pss_sc, b_sb, dst, cq, sc, on_scalar=True):
                q_sb = work.tile([P, 512], bf16, tag="q_sb",
                                 name=f"q_sb_{cq}_{sc}")
                if on_scalar:
                    nc.scalar.add(q_sb[:], pss_sc[:], b_sb[:, cq:cq + 1])
                else:
                    nc.vector.tensor_scalar_add(
                        q_sb[:], pss_sc[:], b_sb[:, cq:cq + 1])
                q_sw = work.tile([P, 512], bf16, tag="q_sw",
                                 name=f"q_sw_{cq}_{sc}")
                nc.vector.stream_shuffle(q_sw[:], q_sb[:], SWAP_MASK)
                p1 = work.tile([P, 512], bf16, tag="p1", name=f"p1_{cq}_{sc}")
                nc.vector.tensor_mul(
                    p1[:], q_sb[:], trigA_sb[:, sc * 512:(sc + 1) * 512])
                p2 = work.tile([P, 512], bf16, tag="p2", name=f"p2_{cq}_{sc}")
                nc.vector.tensor_mul(
                    p2[:], q_sw[:], trigB_sb[:, sc * 512:(sc + 1) * 512])
                nc.vector.tensor_add(
                    dst[:, cq, sc * 512:(sc + 1) * 512], p1[:], p2[:])

            # ---- Q^T cq0 / K^T cq0+cq1 projections + RoPE (prelude) ----
            # kc-outer so each xT chunk is consumed as soon as its DMA lands.
            # 4 sequence tiles accumulate at once: 2 from ps_proj, 2 from
            # ps_sT (borrowed; attention hasn't started).
            for w_sb, b_sb, dst, cq in (
                (wq_sb, qb_sb, QT_rot, 0),
                (wk_sb, kb_sb, KT_rot, 0),
                (wk_sb, kb_sb, KT_rot, 1),
            ):
                pss = []
                for sc in range(4):
                    pool = ps_proj if sc < 2 else ps_sT
                    tag = "proj" if sc < 2 else "sT"
                    pss.append(pool.tile([P, 512], fp32, tag=tag,
                                         name=f"pss_{cq}_{sc}_{dst is KT_rot}"))
                for kc in range(NKC):
                    for sc in range(4):
                        nc.tensor.matmul(
                            pss[sc][:],
                            w_sb[:, kc, cq * P:(cq + 1) * P],
                            xT_sb[:, kc, sc * 512:(sc + 1) * 512],
                            start=(kc == 0), stop=(kc == NKC - 1),
                        )
                for sc in range(4):
                    rope_chain(pss[sc], b_sb, dst, cq, sc)

            # ---- V projection chunk (natural layout, scaled by exp(mask)) ----
            def v_chunk(sc):
                ps = ps_proj.tile([P, HPC * HD], fp32, tag="proj",
                                  name=f"vp_{sc}")
                for kc in range(NKC):
                    nc.tensor.matmul(
                        ps[:],
                        xT_sb[:, kc, sc * P:(sc + 1) * P],
                        wv_sb[:, kc, :],
                        start=(kc == 0), stop=(kc == NKC - 1),
                    )
                # per head: columns 0..63 = V * exp(mask), column 64 = exp(mask)
                vdst = V_aug[:, sc, :].rearrange("p (h x) -> p h x", h=HPC)
                nc.scalar.mul(
                    vdst[:, :, 0:HD],
                    ps[:].rearrange("p (h x) -> p h x", h=HPC),
                    em_sb[:, sc:sc + 1],
                )
                nc.vector.tensor_copy(
                    vdst[:, :, HD:HD + 1],
                    em_sb[:, sc:sc + 1, None].to_broadcast([P, HPC, 1]),
                )

            for sc in range(10):
                v_chunk(sc)

            # deferred work executed inside attention iterations:
            #  - remaining V chunks (10..15) early in (qt=0, hp=0); each is
            #    produced >=8 iterations before its attn_v consumer
            #  - Q cq=1 projection for tile qt during (qt, hp=0)
            defer_pss = {}

            def defer_v(it):
                if it < 6:
                    v_chunk(10 + it)

            def defer_q1(qt, m):
                # 8 matmuls (kc 0..7) over the first 8 iterations + rope
                if m < NKC:
                    kc = m
                    if kc == 0:
                        defer_pss[qt] = ps_proj.tile(
                            [P, 512], fp32, tag="proj", name=f"ip_{qt}")
                    nc.tensor.matmul(
                        defer_pss[qt][:],
                        wq_sb[:, kc, P:2 * P],
                        xT_sb[:, kc, qt * 512:(qt + 1) * 512],
                        start=(kc == 0), stop=(kc == NKC - 1),
                    )
                if m == NKC:
                    rope_chain(defer_pss[qt], qb_sb, QT_rot, 1, qt,
                               on_scalar=False)

            # ---- attention: q-tile outer, head pair inner ----
            oT_norm = const.tile([P, 2, S], bf16)   # normalized o^T, heads packed
            cc_ain = dram.tile([8, P, 2, 512], bf16, name="cc_ain")
            cc_aout = dram.tile([8, P, 2, 512], bf16, name="cc_aout")
            rs_fold = const.tile([P, NQT, 2, 2, 4], bf16)  # row sums [qt, hp, j]

            def oproj(qt):
                # ship this q-tile's normalized heads to the exchange buffer;
                # shards 4b+qt of both batch halves get a copy so the send
                # side is core-independent (peers outside the group ignore it)
                nc.scalar.dma_start(
                    cc_ain[qt], oT_norm[:, :, qt * 512:(qt + 1) * 512])
                nc.scalar.dma_start(
                    cc_ain[4 + qt], oT_norm[:, :, qt * 512:(qt + 1) * 512])

            for qt in range(NQT):
                for hp in range(2):
                    oTs = [ps_o.tile([HD + 1, 512], fp32, tag="oT",
                                     name=f"oT_{qt}_{hp}_{j}")
                           for j in range(2)]

                    def attn_v(kb, at_kb):
                        for j in range(2):
                            h = 2 * hp + j
                            nc.tensor.matmul(
                                oTs[j][:],
                                V_aug[:, kb, h * (HD + 1):(h + 1) * (HD + 1)],
                                at_kb[:, j, :],
                                start=(kb == 0), stop=(kb == NSC - 1),
                            )

                    # attnV software-pipelined two steps behind the exp
                    # stream so the PE never waits on the current tile's exp
                    pend = []
                    for kb in range(NSC):
                        sT = ps_sT.tile([P, 2, 512], fp32, tag="sT")
                        # the two heads occupy partitions 0-63 / 64-127, so
                        # the two K=64 score matmuls run concurrently
                        for j in range(2):
                            nc.tensor.matmul(
                                sT[:, j, :],
                                KT_rot[64 * j:64 * j + 64, hp, kb * P:(kb + 1) * P],
                                QT_rot[64 * j:64 * j + 64, hp,
                                       qt * 512:(qt + 1) * 512],
                                start=True, stop=True,
                            )
                        at = attp.tile([P, 2, 512], bf16, tag="attnT")
                        nc.scalar.activation(
                            at[:], sT[:], mybir.ActivationFunctionType.Exp,
                            scale=0.125)
                        if qt == 0:
                            defer_v(16 * hp + kb)
                        if hp == 0:
                            defer_q1(qt, kb)
                        pend.append((kb, at))
                        if len(pend) > 2:
                            attn_v(*pend.pop(0))
                    for p_ in pend:
                        attn_v(*p_)

                    # stage the head values off PSUM (frees the banks) and
                    # fold the row sums into [128, 4] lanes via DMA
                    st_un = {}
                    for j in range(2):
                        st = stun.tile([HD + 1, 512], bf16, tag="st_un",
                                       name=f"st_un_{qt}_{hp}_{j}")
                        nc.vector.tensor_copy(st[:], oTs[j][:])
                        nc.sync.dma_start(
                            rs_fold[:, qt, hp, j, :], st[HD:HD + 1, :])
                        st_un[j] = st
                    # reciprocal on all 128 lanes at once, then broadcast
                    # back via DRAM on the gpsimd queue
                    rc = work.tile([P, 2, 4], bf16, tag="rc2",
                                   name=f"rc_{qt}_{hp}")
                    with nc.allow_low_precision(
                            reason="softmax scale in bf16 is within budget"):
                        nc.vector.reciprocal(rc[:], rs_fold[:, qt, hp, :, :])
                    rcd = dram_rc.tile([2, 512], bf16, tag="rcd",
                                       name=f"rcd_{qt}_{hp}")
                    for j in range(2):
                        nc.sync.dma_start(rcd[j:j + 1, :], rc[:, j, :])
                    for j in range(2):
                        pbase = 64 * j
                        rb = work.tile([HD, 512], bf16, tag="rbcast")
                        rsrc = rcd[j:j + 1, :]
                        nc.sync.dma_start(
                            rb[:],
                            bass.AP(rsrc.tensor, rsrc.offset, [[0, HD], [1, 512]]))
                        stage = work.tile([HD, 512], bf16, tag="stage")
                        nc.vector.tensor_mul(stage[:], st_un[j][0:HD, :], rb[:])
                        nc.sync.dma_start(
                            oT_norm[pbase:pbase + 64, hp,
                                    qt * 512:(qt + 1) * 512],
                            stage[:])
                oproj(qt)

            # ---- exchange q-slices for head-groups, then full O-proj ----
            nc.gpsimd.collective_compute(
                "AllToAll", mybir.AluOpType.bypass,
                replica_groups=[[0, 1, 2, 3, 4, 5, 6, 7]],
                ins=[cc_ain.opt()], outs=[cc_aout.opt()],
            )
            # chunk s holds rank s's q-slice for my rows; my group's chunks
            # are [4b, 4b+4) — select them with predicated gathers
            a2a_sb = const.tile([P, 4, 2, 512], bf16)
            pid_sy = nc.sync.partition_id()
            pid_sc = nc.scalar.partition_id()
            for s in range(8):
                eng, pid = (nc.sync, pid_sy) if s % 2 == 0 else (nc.scalar, pid_sc)
                cond = (pid >= 4) if s >= 4 else (pid < 4)
                eng.dma_start(a2a_sb[:, s % 4, :, :], cc_aout[s, :, :, :],
                              cond=cond)
            for qs in range(4):
                o_sb = work.tile([P, DIM], bf16, tag="o_sb", name=f"osb_{qs}")
                for dc in range(2):
                    ps = ps_proj.tile([P, 512], fp32, tag="proj",
                                      name=f"op_{qs}_{dc}")
                    for gp in range(4):
                        for c in range(2):
                            nc.tensor.matmul(
                                ps[:],
                                a2a_sb[:, gp, c, qs * P:(qs + 1) * P],
                                wo_sb[:, gp * 2 + c, dc * 512:(dc + 1) * 512],
                                start=(gp == 0 and c == 0),
                                stop=(gp == 3 and c == 1),
                            )
                    nc.vector.tensor_copy(
                        o_sb[:, dc * 512:(dc + 1) * 512], ps[:])
                nc.sync.dma_start(out[qs * P:(qs + 1) * P, :], o_sb[:])

    nc.compile()
    return nc


def _host_prep(x, pos, mask, wq_kernel, wq_bias, wk_kernel, wk_bias,
               wv_kernel, wv_bias, wo_kernel, wo_bias):
    """Build per-core in_maps for the 8 cores."""
    perm = np.array([(j // 2) if j % 2 == 0 else (j // 2 + 32)
                     for j in range(HD)])
    half = HD // 2
    freqs = (10000.0 ** (-np.linspace(0.0, 1.0, half, endpoint=False))).astype(np.float64)

    bf = ml_dtypes.bfloat16
    in_maps = []
    for c in range(8):
        b, g = c // 4, c % 4
        H = list(range(HPC * g, HPC * g + HPC))

        theta = pos[b].astype(np.float64)[:, None] * freqs[None, :]  # [S, 32]
        cos = np.cos(theta).astype(np.float32)
        sin = np.sin(theta).astype(np.float32)
        trigA = np.empty((P, S), np.float32)
        trigB = np.empty((P, S), np.float32)
        for r in range(P):
            j = r % HD
            i = j // 2
            trigA[r] = cos[:, i]
            trigB[r] = (-sin[:, i]) if j % 2 == 0 else sin[:, i]

        def permute_w(wk_):  # [D, H, hd] -> [D, 4*64] with rope-pair row order
            wsel = wk_[:, H, :][:, :, perm]          # [D, 4, 64]
            return np.ascontiguousarray(wsel.reshape(DIM, HPC * HD))

        def permute_b(bias):  # [H, hd] -> [128, 2]
            bsel = bias[H][:, perm]                  # [4, 64]
            return np.ascontiguousarray(bsel.reshape(2, P).T)

        emv = np.exp(mask[b, 0, 0].astype(np.float32))  # [S]

        in_maps.append({
            "xT": np.ascontiguousarray(x[b].T).astype(bf),
            "wq": permute_w(wq_kernel).astype(bf),
            "wk": permute_w(wk_kernel).astype(bf),
            "wv": np.ascontiguousarray(
                wv_kernel[:, H, :].reshape(DIM, HPC * HD)).astype(bf),
            "wo": np.ascontiguousarray(
                wo_kernel.reshape(4, 2, 2, HD, DIM)
                .transpose(2, 3, 0, 1, 4).reshape(P, 8 * DIM)).astype(bf),
            "trigA": trigA.astype(bf),
            "trigB": trigB.astype(bf),
            "qbias": permute_b(wq_bias),
            "kbias": permute_b(wk_bias),
            "em": np.ascontiguousarray(emv.reshape(NSC, P).T),
        })
    return in_maps


def kernel(x, pos, mask, wq_kernel, wq_bias, wk_kernel, wk_bias,
           wv_kernel, wv_bias, wo_kernel, wo_bias):
    x, pos, mask = np.asarray(x), np.asarray(pos), np.asarray(mask)
    wq_kernel, wq_bias = np.asarray(wq_kernel), np.asarray(wq_bias)
    wk_kernel, wk_bias = np.asarray(wk_kernel), np.asarray(wk_bias)
    wv_kernel, wv_bias = np.asarray(wv_kernel), np.asarray(wv_bias)
    wo_kernel, wo_bias = np.asarray(wo_kernel), np.asarray(wo_bias)
    if "nc" not in _CACHE:
        _CACHE["nc"] = _build()
    nc = _CACHE["nc"]

    in_maps = _host_prep(x, pos, mask, wq_kernel, wq_bias, wk_kernel, wk_bias,
                         wv_kernel, wv_bias, wo_kernel, wo_bias)
    res = bass_utils.run_bass_kernel_spmd(
        nc, in_maps, core_ids=list(range(8)))

    final_bias = (wo_bias.astype(np.float64)
                  + np.einsum("hd,hdo->o", wv_bias.astype(np.float64),
                              wo_kernel.astype(np.float64))).astype(np.float32)

    outs = []
    for b in range(B):
        rows = np.concatenate(
            [np.asarray(res.results[4 * b + g]["out"]).astype(np.float32)
             for g in range(4)], axis=0)
        outs.append(rows + final_bias[None, :])
    return np.stack(outs, axis=0)
